# revision 30
# baseline (speedup 1.0000x reference)
"""Multi-head attention (B=4, N=2048, D=1024, H=16) on 8 Trainium2 NeuronCores.

Sharding: core = (batch b = core//2, head-group g = core%2 of 8 heads).
Each core computes qkv + attention for its 8 heads and a *partial* output
projection over its 512 features; the host sums the two partials per batch
and adds the bias (the tensor-parallel unshard).

All matmuls run in fp32r (TF32-like, full PE speed at moving dim >=256).
Scores are computed transposed (S^T[m,n]: keys on partitions) so softmax
needs no on-chip transpose; a ones-column appended to v yields the softmax
denominators inside the same PE accumulation as attn@v.

Emission is software-pipelined for the ACT engine (exp is the per-core
roofline: 33.5M elements at 1 elem/lane/cycle): a short prelude computes
kT, v and qT(chunk 0); per m-pair the attn@v matmuls of iteration i-1 are
emitted between the score matmuls so the PE never blocks the next exp, and
qT for chunk j+1 is produced during chunk j's attention.
"""
import sys

sys.path.insert(0, '/opt/trn_rl_repo')

import numpy as np

import concourse.bass as bass  # noqa: F401  (registers engines)
import concourse.mybir as mybir
import concourse.tile as tile
from concourse import bacc
from concourse.bass_utils import run_bass_kernel_spmd

dt = mybir.dt

B = 4
N = 2048          # sequence length
D = 1024          # d_model
NH = 16           # total heads
HD = 64           # head dim
NHC = 8           # heads per core
DC = NHC * HD     # 512 features per core
SCALE = HD ** -0.5

P = 128           # partitions
BG_INTERLEAVE = False
KB = D // P       # 8 k-blocks
NCH = N // 512    # 4 n-chunks of 512
MT = N // P       # 16 m-tiles of 128
DB = DC // P      # 4 d'-blocks / c-blocks


def build_program(debug=False):
    nc = bacc.Bacc("TRN2", target_bir_lowering=False, debug=False,
                   enable_asserts=False, num_devices=8)

    xT = nc.dram_tensor("xT", [D, N], dt.float32, kind="ExternalInput")
    wqT = nc.dram_tensor("wqT", [D, DC], dt.float32, kind="ExternalInput")
    wkT = nc.dram_tensor("wkT", [D, DC], dt.float32, kind="ExternalInput")
    wvT = nc.dram_tensor("wvT", [D, DC], dt.float32, kind="ExternalInput")
    wpT = nc.dram_tensor("wpT", [DC, D], dt.float32, kind="ExternalInput")
    out = nc.dram_tensor("out", [N, D], dt.float32, kind="ExternalOutput")

    f32r = dt.float32r
    f32 = dt.float32
    Exp = mybir.ActivationFunctionType.Exp
    MULT = mybir.AluOpType.mult
    DIV = mybir.AluOpType.divide

    with tile.TileContext(nc) as tc:
        with tc.tile_pool(name="persist", bufs=1) as persist, \
             tc.tile_pool(name="wq", bufs=1) as wq_pool, \
             tc.tile_pool(name="qTc", bufs=2) as qT_pool, \
             tc.tile_pool(name="xw", bufs=2) as xw_pool, \
             tc.tile_pool(name="ps_S", bufs=3, space="PSUM") as ps_S, \
             tc.tile_pool(name="ps_o", bufs=1, space="PSUM") as ps_o:

            # ---- persistent SBUF tensors ----
            kT_sb = persist.tile([P, DB, N], f32r, tag="kT")
            # v with a ones column per head: [m-part, m-tile, head, 65]
            v_sb = persist.tile([P, MT, NHC, HD + 1], f32r, tag="v")
            ones_sb = persist.tile([P, HD], f32r, tag="ones")

            wq_sb = wq_pool.tile([P, KB, DC], f32r, tag="wq")
            nc.sync.dma_start(
                wq_sb[:], wqT.ap().rearrange("(kb p) d -> p kb d", p=P).bitcast(f32r))
            nc.vector.memset(v_sb[:].bitcast(f32), 1.0)
            nc.vector.memset(ones_sb[:].bitcast(f32), 1.0)

            def load_xw(j, label):
                xw = xw_pool.tile([P, KB, 512], f32r, tag="xw",
                                  name=f"xw_{label}")
                nc.sync.dma_start(
                    xw[:],
                    xT.ap()[:, j * 512:(j + 1) * 512]
                    .rearrange("(kb p) n -> p kb n", p=P).bitcast(f32r))
                return xw

            def emit_proj_tiles(xw, w_sb, dst_fn, lbl):
                """q/k projection for one 512-window: 4 d'-blocks."""
                for db in range(DB):
                    pq = ps_S.tile([P, 1024], f32, tag="S",
                                   name=f"pq_{lbl}_{db}")[:, 0:512]
                    for kb in range(KB):
                        nc.tensor.matmul(
                            pq[:],
                            lhsT=w_sb[:, kb, db * P:(db + 1) * P],
                            rhs=xw[:, kb, :],
                            start=(kb == 0), stop=(kb == KB - 1))
                    nc.vector.tensor_copy(out=dst_fn(db), in_=pq[:])

            def emit_v_window(xw, w, wv_sb):
                """v for the 4 m-tiles of window w."""
                for mc in range(4):
                    m = w * 4 + mc
                    pv = ps_S.tile([P, 1024], f32, tag="S",
                                   name=f"pv{m}")[:, 0:512]
                    for kb in range(KB):
                        nc.tensor.matmul(
                            pv[:],
                            lhsT=xw[:, kb, mc * P:(mc + 1) * P],
                            rhs=wv_sb[:, kb, :],
                            start=(kb == 0), stop=(kb == KB - 1))
                    nc.vector.tensor_copy(
                        out=v_sb[:, m, :, 0:HD],
                        in_=pv[:].rearrange("p (h d) -> p h d", h=NHC))

            qT_tiles = [None] * NCH

            def emit_qT_chunk(j):
                qt = qT_pool.tile([P, DB, 512], f32r, tag="qTc", name=f"qT{j}")
                xwq = load_xw(j, f"q{j}")
                emit_proj_tiles(xwq, wq_sb, lambda db: qt[:, db, :], f"q{j}")
                qT_tiles[j] = qt

            # attention-phase pools (entered before wkv so the wkv pool can
            # be popped in stack order at the end of chunk 0)
            expS_scope = tc.tile_pool(name="expS", bufs=5)
            expS_pool = expS_scope.__enter__()
            at_scope = tc.tile_pool(name="at", bufs=2)
            at_pool = at_scope.__enter__()
            small_scope = tc.tile_pool(name="small", bufs=1)
            small_pool = small_scope.__enter__()
            out_scope = tc.tile_pool(name="outsb", bufs=2)
            out_pool = out_scope.__enter__()

            # ---- prelude: kT + v + qT for window/chunk 0 ----
            wkv_scope = tc.tile_pool(name="wkv", bufs=1)
            wkv_pool = wkv_scope.__enter__()
            wk_sb = wkv_pool.tile([P, KB, DC], f32r, tag="wk")
            wv_sb = wkv_pool.tile([P, KB, DC], f32r, tag="wv")
            nc.sync.dma_start(
                wk_sb[:], wkT.ap().rearrange("(kb p) d -> p kb d", p=P).bitcast(f32r))
            nc.sync.dma_start(
                wv_sb[:], wvT.ap().rearrange("(kb p) d -> p kb d", p=P).bitcast(f32r))

            xw0 = load_xw(0, "kv0")
            emit_proj_tiles(
                xw0, wk_sb,
                lambda db: kT_sb[:, db, 0:512], "k0")
            emit_v_window(xw0, 0, wv_sb)
            emit_qT_chunk(0)

            def emit_kv_window(w):
                xw = load_xw(w, f"kv{w}")
                emit_proj_tiles(
                    xw, wk_sb,
                    lambda db, w=w: kT_sb[:, db, w * 512:(w + 1) * 512],
                    f"k{w}")
                emit_v_window(xw, w, wv_sb)

            if debug:
                dbg_qT = nc.dram_tensor("dbg_qT", [P, DB, 512], f32, kind="ExternalOutput")
                dbg_kT = nc.dram_tensor("dbg_kT", [P, DB, N], f32, kind="ExternalOutput")
                dbg_v = nc.dram_tensor("dbg_v", [P, MT, NHC, HD + 1], f32, kind="ExternalOutput")
                dbg_at = nc.dram_tensor("dbg_at", [P, DB, 512], f32, kind="ExternalOutput")
                nc.sync.dma_start(dbg_qT.ap(), qT_tiles[0][:].bitcast(f32))

            # ---- attention + projection, per n-chunk ----
            # All score/qT/proj/bcp PSUM traffic shares one 3-deep ring of
            # [128,1024] tiles (6 banks); attn@v accumulators get 2 banks.
            at_tiles = [None] * NCH

            def emit_qT_thunks(j):
                """qT(j) emission as small PE thunks (ring-pool psum)."""
                qt = qT_pool.tile([P, DB, 512], f32r, tag="qTc", name=f"qT{j}")
                qT_tiles[j] = qt
                xwq = load_xw(j, f"q{j}")
                thunks = []
                box = [None]
                for db in range(DB):
                    def mm_t(db, kb0):
                        if kb0 == 0:
                            box[0] = ps_S.tile([P, 1024], f32, tag="S",
                                               name=f"pqt{db}")
                        for kb in (kb0, kb0 + 1):
                            nc.tensor.matmul(
                                box[0][:, 0:512],
                                lhsT=wq_sb[:, kb, db * P:(db + 1) * P],
                                rhs=xwq[:, kb, :],
                                start=(kb == 0), stop=(kb == KB - 1))
                    for kb0 in range(0, KB, 2):
                        thunks.append(lambda db=db, kb0=kb0: mm_t(db, kb0))
                    def cp_t(db=db, qt=qt):
                        nc.vector.tensor_copy(out=qt[:, db, :],
                                              in_=box[0][:, 0:512])
                    thunks.append(cp_t)
                return thunks

            def emit_proj_thunks(j):
                """Projection of chunk j as small PE thunks (ring psum)."""
                at_j = at_tiles[j]
                thunks = []
                box = [None]
                for ns in range(4):
                    for ec in range(2):
                        def mm_t(ns, ec, kb0):
                            if kb0 == 0:
                                box[0] = ps_S.tile([P, 1024], f32, tag="S",
                                                   name=f"ppt{ns}_{ec}")
                            for cb in (kb0, kb0 + 1):
                                nc.tensor.matmul(
                                    box[0][:, 0:512],
                                    lhsT=at_j[:, cb, ns * P:(ns + 1) * P],
                                    rhs=wp_box[0][:, cb, ec * 512:(ec + 1) * 512],
                                    start=(cb == 0), stop=(cb == DB - 1))
                        for kb0 in range(0, DB, 2):
                            thunks.append(
                                lambda ns=ns, ec=ec, kb0=kb0: mm_t(ns, ec, kb0))
                        def cp_t(ns=ns, ec=ec):
                            osb = out_pool.tile([P, 512], f32, tag="osb",
                                                name=f"osb{ns}_{ec}")
                            nc.vector.tensor_copy(out=osb[:], in_=box[0][:, 0:512])
                            nc.sync.dma_start(
                                out.ap()[j * 512 + ns * P:j * 512 + (ns + 1) * P,
                                         ec * 512:(ec + 1) * 512],
                                osb[:])
                        thunks.append(cp_t)
                return thunks

            wp_box = [None]

            for j in range(NCH):
                if j == 1:
                    wp_scope = tc.tile_pool(name="wp", bufs=1)
                    wp_pool = wp_scope.__enter__()
                    wp_box.append(wp_scope)  # keep scope alive
                    wp_sb = wp_pool.tile([P, DB, D], f32r, tag="wp")
                    nc.sync.dma_start(
                        wp_sb[:],
                        wpT.ap().rearrange("(cb p) e -> p cb e", p=P).bitcast(f32r))
                    wp_box[0] = wp_sb
                qt = qT_tiles[j]
                at = at_pool.tile([P, DB, 512], f32r, tag="at", name=f"at{j}")
                at_tiles[j] = at

                background = []
                if j + 1 < NCH:
                    background += emit_qT_thunks(j + 1)
                if j >= 1:
                    background += emit_proj_thunks(j - 1)
                bg_pos = [0]

                def emit_bg():
                    if bg_pos[0] < len(background):
                        background[bg_pos[0]]()
                        bg_pos[0] += 1

                def emit_S(p, h, i):
                    rsl = slice(h * HD, (h + 1) * HD)
                    S = ps_S.tile([P, 1024], f32, tag="S", name=f"S{h}_{i}")
                    for half in range(2):
                        m = 2 * i + half
                        nc.tensor.matmul(
                            S[:, half * 512:(half + 1) * 512],
                            lhsT=kT_sb[rsl, p, m * P:(m + 1) * P],
                            rhs=qt[rsl, p, :],
                            start=True, stop=True)
                    return S

                def emit_epilogue(po_t, p, h):
                    oT = small_pool.tile([HD + 1, 512], f32, tag=f"oT{h}",
                                         name=f"oT{h}")
                    nc.vector.tensor_copy(out=oT[:], in_=po_t[0:HD + 1, :])
                    rcp = small_pool.tile([HD + 1, 512], f32r, tag="rcp",
                                          name="rcp")
                    with nc.allow_low_precision(reason="softmax recip to f32r"):
                        nc.vector.reciprocal(rcp[HD:HD + 1, :],
                                             oT[HD:HD + 1, :])
                    bcp = ps_S.tile([P, 1024], f32, tag="S", name=f"bcp{h}")
                    nc.tensor.matmul(bcp[0:HD, 0:512],
                                     lhsT=ones_sb[HD:HD + 1, :],
                                     rhs=rcp[HD:HD + 1, :],
                                     start=True, stop=True)
                    if h == 0:
                        nc.vector.tensor_tensor(
                            out=at[0:HD, p, :], in0=oT[0:HD, :],
                            in1=bcp[0:HD, 0:512], op=MULT)
                    else:
                        nc.vector.tensor_tensor(
                            out=oT[0:HD, :], in0=oT[0:HD, :],
                            in1=bcp[0:HD, 0:512], op=MULT)
                        nc.sync.dma_start(at[HD:P, p, :],
                                          oT[0:HD, :].bitcast(f32r))

                for p in range(DB):  # head pair p -> heads 2p, 2p+1
                    po = [ps_o.tile([P, 512], f32, tag="o", name=f"po{h}")
                          for h in range(2)]
                    steps = [(h, i) for h in range(2) for i in range(MT // 2)]
                    eS_q = {}
                    AV_LAG = 2

                    def emit_av(idx2):
                        ph, pi = steps[idx2]
                        eSp = eS_q.pop((ph, pi))
                        for half in range(2):
                            m = 2 * pi + half
                            nc.tensor.matmul(
                                po[ph][0:HD + 1, :],
                                lhsT=v_sb[:, m, 2 * p + ph, :],
                                rhs=eSp[:, half * 512:(half + 1) * 512],
                                start=(m == 0), stop=(m == MT - 1))

                    S_next = emit_S(p, *steps[0])
                    for idx, (h, i) in enumerate(steps):
                        S_cur = S_next
                        eS = expS_pool.tile([P, 1024], f32r, tag="e",
                                            name=f"eS{h}_{i}")
                        nc.scalar.activation(eS[:], S_cur[:], Exp, scale=SCALE)
                        eS_q[(h, i)] = eS
                        if j == 0 and p == 0 and h == 0 and i in (1, 3, 5):
                            emit_kv_window(i // 2 + 1)
                        if idx + 1 < len(steps):
                            S_next = emit_S(p, *steps[idx + 1])
                        if idx >= AV_LAG:
                            emit_av(idx - AV_LAG)
                        if BG_INTERLEAVE:
                            emit_bg()
                            if len(background) - bg_pos[0] >                                     (len(steps) - idx) * (DB - p):
                                emit_bg()

                    for idx2 in range(len(steps) - AV_LAG, len(steps)):
                        emit_av(idx2)
                    emit_epilogue(po[0], p, 0)
                    emit_epilogue(po[1], p, 1)

                while bg_pos[0] < len(background):
                    emit_bg()
                if j == 0:
                    wkv_scope.__exit__(None, None, None)

                if debug and j == 0:
                    nc.sync.dma_start(dbg_at.ap(), at[:].bitcast(f32))

            # final chunk's projection
            for t in emit_proj_thunks(NCH - 1):
                t()

            if len(wp_box) > 1:
                wp_box[1].__exit__(None, None, None)
            out_scope.__exit__(None, None, None)
            small_scope.__exit__(None, None, None)
            at_scope.__exit__(None, None, None)
            expS_scope.__exit__(None, None, None)

    nc.compile()
    return nc


_CACHE: dict = {}


def _get_program():
    if "nc" not in _CACHE:
        _CACHE["nc"] = build_program()
    return _CACHE["nc"]


def make_in_maps(x, w_qkv, w_proj):
    """Host-side sharding: per-core input dict."""
    x = np.ascontiguousarray(np.asarray(x, dtype=np.float32))
    w_qkv = np.asarray(w_qkv, dtype=np.float32)
    w_proj = np.asarray(w_proj, dtype=np.float32)
    in_maps = []
    for core in range(8):
        b, g = divmod(core, 2)
        gsl = slice(g * DC, (g + 1) * DC)
        in_maps.append({
            "xT": np.ascontiguousarray(x[b].T),                       # [D, N]
            "wqT": np.ascontiguousarray(w_qkv[0 * D:1 * D][gsl].T),   # [D, DC]
            "wkT": np.ascontiguousarray(w_qkv[1 * D:2 * D][gsl].T),
            "wvT": np.ascontiguousarray(w_qkv[2 * D:3 * D][gsl].T),
            "wpT": np.ascontiguousarray(w_proj[:, gsl].T),            # [DC, D]
        })
    return in_maps


def run(x, w_qkv, w_proj, b_proj, **spmd_kwargs):
    nc = _get_program()
    in_maps = make_in_maps(x, w_qkv, w_proj)
    res = run_bass_kernel_spmd(nc, in_maps, list(range(8)), **spmd_kwargs)
    b_proj = np.asarray(b_proj, dtype=np.float32)
    outp = np.empty((B, N, D), dtype=np.float32)
    for b in range(B):
        outp[b] = (res.results[2 * b]["out"] + res.results[2 * b + 1]["out"]
                   + b_proj[None, :])
    return outp, res


def kernel(x, w_qkv, w_proj, b_proj):
    outp, _ = run(x, w_qkv, w_proj, b_proj)
    return outp


# revision 31
# speedup vs baseline: 1.2371x; 1.2371x over previous
"""Multi-head attention (B=4, N=2048, D=1024, H=16) on 8 Trainium2 NeuronCores.

Sharding: core = (batch b = core//2, head-group g = core%2 of 8 heads).
Each core computes qkv + attention for its 8 heads and a *partial* output
projection over its 512 features; the host sums the two partials per batch
and adds the bias (the tensor-parallel unshard).

All matmuls run in fp32r (TF32-like, full PE speed at moving dim >=256).
Scores are computed transposed (S^T[m,n]: keys on partitions) so softmax
needs no on-chip transpose; a ones-column appended to v yields the softmax
denominators inside the same PE accumulation as attn@v.

Emission is software-pipelined for the ACT engine (exp is the per-core
roofline: 33.5M elements at 1 elem/lane/cycle): a short prelude computes
kT, v and qT(chunk 0); per m-pair the attn@v matmuls of iteration i-1 are
emitted between the score matmuls so the PE never blocks the next exp, and
qT for chunk j+1 is produced during chunk j's attention.
"""
import sys

sys.path.insert(0, '/opt/trn_rl_repo')

import numpy as np

import concourse.bass as bass  # noqa: F401  (registers engines)
import concourse.mybir as mybir
import concourse.tile as tile
from concourse import bacc
from concourse.bass_utils import run_bass_kernel_spmd

dt = mybir.dt

B = 4
N = 2048          # sequence length
D = 1024          # d_model
NH = 16           # total heads
HD = 64           # head dim
NHC = 8           # heads per core
DC = NHC * HD     # 512 features per core
SCALE = HD ** -0.5

P = 128           # partitions
BG_INTERLEAVE = True
KB = D // P       # 8 k-blocks
NCH = N // 512    # 4 n-chunks of 512
MT = N // P       # 16 m-tiles of 128
DB = DC // P      # 4 d'-blocks / c-blocks


def build_program(debug=False):
    nc = bacc.Bacc("TRN2", target_bir_lowering=False, debug=False,
                   enable_asserts=False, num_devices=8)

    xT = nc.dram_tensor("xT", [D, N], dt.float32, kind="ExternalInput")
    wqT = nc.dram_tensor("wqT", [D, DC], dt.float32, kind="ExternalInput")
    wkT = nc.dram_tensor("wkT", [D, DC], dt.float32, kind="ExternalInput")
    wvT = nc.dram_tensor("wvT", [D, DC], dt.float32, kind="ExternalInput")
    wpT = nc.dram_tensor("wpT", [DC, D], dt.float32, kind="ExternalInput")
    out = nc.dram_tensor("out", [N, D], dt.float32, kind="ExternalOutput")

    f32r = dt.float32r
    f32 = dt.float32
    Exp = mybir.ActivationFunctionType.Exp
    MULT = mybir.AluOpType.mult
    DIV = mybir.AluOpType.divide

    with tile.TileContext(nc) as tc:
        with tc.tile_pool(name="persist", bufs=1) as persist, \
             tc.tile_pool(name="wq", bufs=1) as wq_pool, \
             tc.tile_pool(name="qTc", bufs=2) as qT_pool, \
             tc.tile_pool(name="xw", bufs=2) as xw_pool, \
             tc.tile_pool(name="ps_S", bufs=2, space="PSUM") as ps_S, \
             tc.tile_pool(name="ps_bg", bufs=2, space="PSUM") as ps_bg, \
             tc.tile_pool(name="ps_o", bufs=1, space="PSUM") as ps_o:

            # ---- persistent SBUF tensors ----
            kT_sb = persist.tile([P, DB, N], f32r, tag="kT")
            # v with a ones column per head: [m-part, m-tile, head, 65]
            v_sb = persist.tile([P, MT, NHC, HD + 1], f32r, tag="v")
            ones_sb = persist.tile([P, HD], f32r, tag="ones")

            wq_sb = wq_pool.tile([P, KB, DC], f32r, tag="wq")
            nc.sync.dma_start(
                wq_sb[:], wqT.ap().rearrange("(kb p) d -> p kb d", p=P).bitcast(f32r))
            nc.vector.memset(v_sb[:].bitcast(f32), 1.0)
            nc.vector.memset(ones_sb[:].bitcast(f32), 1.0)

            def load_xw(j, label):
                xw = xw_pool.tile([P, KB, 512], f32r, tag="xw",
                                  name=f"xw_{label}")
                nc.sync.dma_start(
                    xw[:],
                    xT.ap()[:, j * 512:(j + 1) * 512]
                    .rearrange("(kb p) n -> p kb n", p=P).bitcast(f32r))
                return xw

            def emit_proj_tiles(xw, w_sb, dst_fn, lbl):
                """q/k projection for one 512-window: 4 d'-blocks."""
                for db in range(DB):
                    pq = ps_bg.tile([P, 512], f32, tag="bg",
                                    name=f"pq_{lbl}_{db}")
                    for kb in range(KB):
                        nc.tensor.matmul(
                            pq[:],
                            lhsT=w_sb[:, kb, db * P:(db + 1) * P],
                            rhs=xw[:, kb, :],
                            start=(kb == 0), stop=(kb == KB - 1))
                    nc.vector.tensor_copy(out=dst_fn(db), in_=pq[:])

            def emit_v_window(xw, w, wv_sb):
                """v for the 4 m-tiles of window w."""
                for mc in range(4):
                    m = w * 4 + mc
                    pv = ps_bg.tile([P, 512], f32, tag="bg", name=f"pv{m}")
                    for kb in range(KB):
                        nc.tensor.matmul(
                            pv[:],
                            lhsT=xw[:, kb, mc * P:(mc + 1) * P],
                            rhs=wv_sb[:, kb, :],
                            start=(kb == 0), stop=(kb == KB - 1))
                    nc.vector.tensor_copy(
                        out=v_sb[:, m, :, 0:HD],
                        in_=pv[:].rearrange("p (h d) -> p h d", h=NHC))

            qT_tiles = [None] * NCH

            def emit_qT_chunk(j):
                qt = qT_pool.tile([P, DB, 512], f32r, tag="qTc", name=f"qT{j}")
                xwq = load_xw(j, f"q{j}")
                emit_proj_tiles(xwq, wq_sb, lambda db: qt[:, db, :], f"q{j}")
                qT_tiles[j] = qt

            # attention-phase pools (entered before wkv so the wkv pool can
            # be popped in stack order at the end of chunk 0)
            expS_scope = tc.tile_pool(name="expS", bufs=5)
            expS_pool = expS_scope.__enter__()
            at_scope = tc.tile_pool(name="at", bufs=2)
            at_pool = at_scope.__enter__()
            small_scope = tc.tile_pool(name="small", bufs=1)
            small_pool = small_scope.__enter__()
            out_scope = tc.tile_pool(name="outsb", bufs=2)
            out_pool = out_scope.__enter__()

            # ---- prelude: kT + v + qT for window/chunk 0 ----
            wkv_scope = tc.tile_pool(name="wkv", bufs=1)
            wkv_pool = wkv_scope.__enter__()
            wk_sb = wkv_pool.tile([P, KB, DC], f32r, tag="wk")
            wv_sb = wkv_pool.tile([P, KB, DC], f32r, tag="wv")
            nc.sync.dma_start(
                wk_sb[:], wkT.ap().rearrange("(kb p) d -> p kb d", p=P).bitcast(f32r))
            nc.sync.dma_start(
                wv_sb[:], wvT.ap().rearrange("(kb p) d -> p kb d", p=P).bitcast(f32r))

            xw0 = load_xw(0, "kv0")
            emit_proj_tiles(
                xw0, wk_sb,
                lambda db: kT_sb[:, db, 0:512], "k0")
            emit_v_window(xw0, 0, wv_sb)
            emit_qT_chunk(0)

            def emit_kv_window(w):
                xw = load_xw(w, f"kv{w}")
                emit_proj_tiles(
                    xw, wk_sb,
                    lambda db, w=w: kT_sb[:, db, w * 512:(w + 1) * 512],
                    f"k{w}")
                emit_v_window(xw, w, wv_sb)

            if debug:
                dbg_qT = nc.dram_tensor("dbg_qT", [P, DB, 512], f32, kind="ExternalOutput")
                dbg_kT = nc.dram_tensor("dbg_kT", [P, DB, N], f32, kind="ExternalOutput")
                dbg_v = nc.dram_tensor("dbg_v", [P, MT, NHC, HD + 1], f32, kind="ExternalOutput")
                dbg_at = nc.dram_tensor("dbg_at", [P, DB, 512], f32, kind="ExternalOutput")
                nc.sync.dma_start(dbg_qT.ap(), qT_tiles[0][:].bitcast(f32))

            # ---- attention + projection, per n-chunk ----
            # All score/qT/proj/bcp PSUM traffic shares one 3-deep ring of
            # [128,1024] tiles (6 banks); attn@v accumulators get 2 banks.
            at_tiles = [None] * NCH

            def emit_qT_thunks(j):
                """qT(j) emission as small PE thunks (ring-pool psum)."""
                qt = qT_pool.tile([P, DB, 512], f32r, tag="qTc", name=f"qT{j}")
                qT_tiles[j] = qt
                xwq = load_xw(j, f"q{j}")
                thunks = []
                box = [None]
                for db in range(DB):
                    def mm_t(db, kb0):
                        if kb0 == 0:
                            box[0] = ps_bg.tile([P, 512], f32, tag="bg",
                                                name=f"pqt{db}")
                        for kb in (kb0, kb0 + 1):
                            nc.tensor.matmul(
                                box[0][:],
                                lhsT=wq_sb[:, kb, db * P:(db + 1) * P],
                                rhs=xwq[:, kb, :],
                                start=(kb == 0), stop=(kb == KB - 1))
                    for kb0 in range(0, KB, 2):
                        thunks.append(lambda db=db, kb0=kb0: mm_t(db, kb0))
                    def cp_t(db=db, qt=qt):
                        nc.vector.tensor_copy(out=qt[:, db, :],
                                              in_=box[0][:, 0:512])
                    thunks.append(cp_t)
                return thunks

            def emit_proj_thunks(j):
                """Projection of chunk j as small PE thunks (ring psum)."""
                at_j = at_tiles[j]
                thunks = []
                box = [None]
                for ns in range(4):
                    for ec in range(2):
                        def mm_t(ns, ec, kb0):
                            if kb0 == 0:
                                box[0] = ps_bg.tile([P, 512], f32, tag="bg",
                                                    name=f"ppt{ns}_{ec}")
                            for cb in (kb0, kb0 + 1):
                                nc.tensor.matmul(
                                    box[0][:],
                                    lhsT=at_j[:, cb, ns * P:(ns + 1) * P],
                                    rhs=wp_box[0][:, cb, ec * 512:(ec + 1) * 512],
                                    start=(cb == 0), stop=(cb == DB - 1))
                        for kb0 in range(0, DB, 2):
                            thunks.append(
                                lambda ns=ns, ec=ec, kb0=kb0: mm_t(ns, ec, kb0))
                        def cp_t(ns=ns, ec=ec):
                            osb = out_pool.tile([P, 512], f32, tag="osb",
                                                name=f"osb{ns}_{ec}")
                            nc.vector.tensor_copy(out=osb[:], in_=box[0][:])
                            nc.sync.dma_start(
                                out.ap()[j * 512 + ns * P:j * 512 + (ns + 1) * P,
                                         ec * 512:(ec + 1) * 512],
                                osb[:])
                        thunks.append(cp_t)
                return thunks

            wp_box = [None]

            for j in range(NCH):
                if j == 1:
                    wp_scope = tc.tile_pool(name="wp", bufs=1)
                    wp_pool = wp_scope.__enter__()
                    wp_box.append(wp_scope)  # keep scope alive
                    wp_sb = wp_pool.tile([P, DB, D], f32r, tag="wp")
                    nc.sync.dma_start(
                        wp_sb[:],
                        wpT.ap().rearrange("(cb p) e -> p cb e", p=P).bitcast(f32r))
                    wp_box[0] = wp_sb
                qt = qT_tiles[j]
                at = at_pool.tile([P, DB, 512], f32r, tag="at", name=f"at{j}")
                at_tiles[j] = at

                background = []
                if j + 1 < NCH:
                    background += emit_qT_thunks(j + 1)
                if j >= 1:
                    background += emit_proj_thunks(j - 1)
                bg_pos = [0]

                def emit_bg():
                    if bg_pos[0] < len(background):
                        background[bg_pos[0]]()
                        bg_pos[0] += 1

                def emit_S(p, h, i):
                    rsl = slice(h * HD, (h + 1) * HD)
                    S = ps_S.tile([P, 1024], f32, tag="S", name=f"S{h}_{i}")
                    for half in range(2):
                        m = 2 * i + half
                        nc.tensor.matmul(
                            S[:, half * 512:(half + 1) * 512],
                            lhsT=kT_sb[rsl, p, m * P:(m + 1) * P],
                            rhs=qt[rsl, p, :],
                            start=True, stop=True)
                    return S

                def emit_epilogue(po_t, p, h):
                    oT = small_pool.tile([HD + 1, 512], f32, tag=f"oT{h}",
                                         name=f"oT{h}")
                    nc.vector.tensor_copy(out=oT[:], in_=po_t[0:HD + 1, :])
                    rcp = small_pool.tile([HD + 1, 512], f32r, tag="rcp",
                                          name="rcp")
                    with nc.allow_low_precision(reason="softmax recip to f32r"):
                        nc.vector.reciprocal(rcp[HD:HD + 1, :],
                                             oT[HD:HD + 1, :])
                    bcp = ps_bg.tile([P, 512], f32, tag="bg", name=f"bcp{h}")
                    nc.tensor.matmul(bcp[0:HD, :],
                                     lhsT=ones_sb[HD:HD + 1, :],
                                     rhs=rcp[HD:HD + 1, :],
                                     start=True, stop=True)
                    if h == 0:
                        nc.vector.tensor_tensor(
                            out=at[0:HD, p, :], in0=oT[0:HD, :],
                            in1=bcp[0:HD, :], op=MULT)
                    else:
                        nc.vector.tensor_tensor(
                            out=oT[0:HD, :], in0=oT[0:HD, :],
                            in1=bcp[0:HD, :], op=MULT)
                        nc.sync.dma_start(at[HD:P, p, :],
                                          oT[0:HD, :].bitcast(f32r))

                for p in range(DB):  # head pair p -> heads 2p, 2p+1
                    po = [ps_o.tile([P, 512], f32, tag="o", name=f"po{h}")
                          for h in range(2)]
                    steps = [(h, i) for h in range(2) for i in range(MT // 2)]
                    eS_q = {}
                    AV_LAG = 2

                    def emit_av(idx2):
                        ph, pi = steps[idx2]
                        eSp = eS_q.pop((ph, pi))
                        for half in range(2):
                            m = 2 * pi + half
                            nc.tensor.matmul(
                                po[ph][0:HD + 1, :],
                                lhsT=v_sb[:, m, 2 * p + ph, :],
                                rhs=eSp[:, half * 512:(half + 1) * 512],
                                start=(m == 0), stop=(m == MT - 1))

                    S_next = emit_S(p, *steps[0])
                    for idx, (h, i) in enumerate(steps):
                        S_cur = S_next
                        eS = expS_pool.tile([P, 1024], f32r, tag="e",
                                            name=f"eS{h}_{i}")
                        nc.scalar.activation(eS[:], S_cur[:], Exp, scale=SCALE)
                        eS_q[(h, i)] = eS
                        if j == 0 and p == 0 and h == 0 and i in (1, 3, 5):
                            emit_kv_window(i // 2 + 1)
                        if idx + 1 < len(steps):
                            S_next = emit_S(p, *steps[idx + 1])
                        if idx >= AV_LAG:
                            emit_av(idx - AV_LAG)
                        if BG_INTERLEAVE:
                            emit_bg()
                            if len(background) - bg_pos[0] >                                     (len(steps) - idx) * (DB - p):
                                emit_bg()

                    for idx2 in range(len(steps) - AV_LAG, len(steps)):
                        emit_av(idx2)
                    emit_epilogue(po[0], p, 0)
                    emit_epilogue(po[1], p, 1)

                while bg_pos[0] < len(background):
                    emit_bg()
                if j == 0:
                    wkv_scope.__exit__(None, None, None)

                if debug and j == 0:
                    nc.sync.dma_start(dbg_at.ap(), at[:].bitcast(f32))

            # final chunk's projection
            for t in emit_proj_thunks(NCH - 1):
                t()

            if len(wp_box) > 1:
                wp_box[1].__exit__(None, None, None)
            out_scope.__exit__(None, None, None)
            small_scope.__exit__(None, None, None)
            at_scope.__exit__(None, None, None)
            expS_scope.__exit__(None, None, None)

    nc.compile()
    return nc


_CACHE: dict = {}


def _get_program():
    if "nc" not in _CACHE:
        _CACHE["nc"] = build_program()
    return _CACHE["nc"]


def make_in_maps(x, w_qkv, w_proj):
    """Host-side sharding: per-core input dict."""
    x = np.ascontiguousarray(np.asarray(x, dtype=np.float32))
    w_qkv = np.asarray(w_qkv, dtype=np.float32)
    w_proj = np.asarray(w_proj, dtype=np.float32)
    in_maps = []
    for core in range(8):
        b, g = divmod(core, 2)
        gsl = slice(g * DC, (g + 1) * DC)
        in_maps.append({
            "xT": np.ascontiguousarray(x[b].T),                       # [D, N]
            "wqT": np.ascontiguousarray(w_qkv[0 * D:1 * D][gsl].T),   # [D, DC]
            "wkT": np.ascontiguousarray(w_qkv[1 * D:2 * D][gsl].T),
            "wvT": np.ascontiguousarray(w_qkv[2 * D:3 * D][gsl].T),
            "wpT": np.ascontiguousarray(w_proj[:, gsl].T),            # [DC, D]
        })
    return in_maps


def run(x, w_qkv, w_proj, b_proj, **spmd_kwargs):
    nc = _get_program()
    in_maps = make_in_maps(x, w_qkv, w_proj)
    res = run_bass_kernel_spmd(nc, in_maps, list(range(8)), **spmd_kwargs)
    b_proj = np.asarray(b_proj, dtype=np.float32)
    outp = np.empty((B, N, D), dtype=np.float32)
    for b in range(B):
        outp[b] = (res.results[2 * b]["out"] + res.results[2 * b + 1]["out"]
                   + b_proj[None, :])
    return outp, res


def kernel(x, w_qkv, w_proj, b_proj):
    outp, _ = run(x, w_qkv, w_proj, b_proj)
    return outp
